# revision 20
# baseline (speedup 1.0000x reference)
"""Trainium2 Bass kernel for nn_Attention_85074712199827.

Computes, for hidden [1,32,1024], encoder_outputs [32,2048,1024],
W_attn [1024,2048], b_attn [1024], v [1024]:

    h_proj  = hidden[0] @ W_attn[:, :1024].T
    e_proj  = encoder_outputs @ W_attn[:, 1024:].T
    energy  = tanh(e_proj + h_proj[:, None, :] + b_attn)
    att     = energy @ v
    out     = softmax(att, axis=1)          # [32, 2048] float32

Distribution: data-parallel over the batch across 8 NeuronCores (4
batch rows per core); the tiny parameters are replicated.  h_proj +
b_attn (67 MFLOP, 0.04% of the work) is precomputed on the host and
shipped as a [128, 8, 4] bias tensor, so the device only runs the
e_proj pipeline.  Each core runs an independent Bass/Tile program;
results are concatenated on the host.

Per-core schedule: enc streams HBM->SBUF fp32 on the sync HWDGE queue
(loads only -- nothing else shares that ring), is cast to bf16 on the
GpSimd engine (so the cast never queues behind DVE work and input
buffers recycle immediately), and xbar-transposed to [h, s] layout on
the scalar HWDGE queue, with the transposes emitted between the tanh
ACTIVATEs so they dispatch early without head-of-line-blocking them.
The e_proj matmuls alternate PSUM banks (`for hc: for c0`) -- back-to-
back accumulation into the same bank costs ~50ns per matmul.  The
v-dot is NOT done with 1-row PE matmuls: the tanh output is scaled by
v on the scalar engine, the 8 o-chunks are summed on DVE, and a single
ones-vector matmul reduces the 128 partitions.  Softmax runs per batch
row as soon as that row's attention logits are complete, overlapped
with the next row's matmuls.

Self-contained: only environment packages (concourse, numpy, ml_dtypes)
are imported; all shapes/sharding are hardcoded for this problem.
"""

from contextlib import ExitStack

import ml_dtypes
import numpy as np

import concourse.bass as bass
import concourse.tile as tile
from concourse import bacc, mybir

F32 = mybir.dt.float32
BF16 = mybir.dt.bfloat16
AF = mybir.ActivationFunctionType
ADD = mybir.AluOpType.add
P = 128


def build_nc(b_loc=4, s=2048, h=1024, n_cores=8, sb=1024,
             warmup_mm=20, startup_keep=4, keepalive_mm=0,
             first_sb=512, ld_chunk=256):
    n_hc = h // P           # contraction chunks
    n_ot = h // P           # output (o) tiles

    nc = bacc.Bacc("TRN2", target_bir_lowering=False, debug=False,
                   num_devices=n_cores)

    wt = nc.dram_tensor("wt", [h, h], BF16, kind="ExternalInput").ap()
    hbias = nc.dram_tensor("hbias", [P, n_ot, b_loc], F32,
                           kind="ExternalInput").ap()
    v = nc.dram_tensor("v", [h], F32, kind="ExternalInput").ap()
    ones = nc.dram_tensor("ones", [P, 1], BF16, kind="ExternalInput").ap()
    zeros = nc.dram_tensor("zeros", [P, 512], BF16, kind="ExternalInput").ap()
    enc = nc.dram_tensor("enc", [b_loc, s, h], F32, kind="ExternalInput").ap()
    out = nc.dram_tensor("out", [b_loc, s], F32, kind="ExternalOutput").ap()

    with tile.TileContext(nc) as tc, ExitStack() as ctx:
        const = ctx.enter_context(tc.tile_pool(name="const", bufs=1))
        psmall = ctx.enter_context(tc.tile_pool(name="psmall", bufs=1, space="PSUM"))

        # ---- PE warmup: dependency-free matmuls to lift the HAM clock
        # gate to 8/8 while the first enc block is still in flight ----
        wz = const.tile([P, 512], BF16)
        nc.scalar.dma_start(wz[:], zeros)
        for i in range(warmup_mm):
            pw = psmall.tile([P, 512], F32, name="pw", tag="ps")
            nc.tensor.matmul(pw[:], wz[:, :P], wz[:], start=True, stop=True)

        def keepalive(n):
            for _ in range(n):
                pw = psmall.tile([P, 512], F32, name="pw", tag="ps")
                nc.tensor.matmul(pw[:], wz[:, :P], wz[:], start=True, stop=True)

        # ---- weights: We.T arrives [h, h] bf16, streamed first so the
        # first e_proj matmuls unblock asap; then the small constants ----
        wt_bf = const.tile([P, n_hc, h], BF16)
        wt_r = wt.rearrange("(jc p) o -> p jc o", p=P)
        q = n_hc // 2
        for c in range(2):
            nc.scalar.dma_start(
                wt_bf[:, c * q:(c + 1) * q, :],
                wt_r[:, c * q:(c + 1) * q, :])

        hb = const.tile([P, n_ot, b_loc], F32, name="hb")
        nc.scalar.dma_start(hb[:], hbias)

        vt_f = const.tile([P, n_ot], F32)
        nc.scalar.dma_start(vt_f[:], v.rearrange("(oc p) -> p oc", p=P))

        ones_bf = const.tile([P, 1], BF16)
        nc.scalar.dma_start(ones_bf[:], ones)

        # ---- main pipeline pools ----
        inp = ctx.enter_context(tc.tile_pool(name="inp", bufs=4))
        bfp = ctx.enter_context(tc.tile_pool(name="bfp", bufs=8))
        encT_p = ctx.enter_context(tc.tile_pool(name="encT", bufs=2))
        en_p = ctx.enter_context(tc.tile_pool(name="energy", bufs=3))
        tmp_p = ctx.enter_context(tc.tile_pool(name="vtmp", bufs=3))
        acc_p = ctx.enter_context(tc.tile_pool(name="acc", bufs=3))
        row_p = ctx.enter_context(tc.tile_pool(name="rowbuf", bufs=2))
        pe_p = ctx.enter_context(tc.tile_pool(name="psum_e", bufs=2, space="PSUM"))
        pa_p = ctx.enter_context(tc.tile_pool(name="psum_att", bufs=2, space="PSUM"))

        # per-row logits buffers (DVE/ACT accesses must start at partition 0)
        att_rows = [const.tile([1, s], F32, name=f"attrow{b}")
                    for b in range(b_loc)]

        # units: (b, s0, size) — b-major so softmax(b) pipelines.
        # The first units of b=0 are small for a fast pipeline rampup.
        units = []
        for b in range(b_loc):
            if b == 0 and first_sb < sb:
                for s0 in range(0, sb, first_sb):
                    units.append((b, s0, first_sb))
                for s0 in range(sb, s, sb):
                    units.append((b, s0, sb))
            else:
                for s0 in range(0, s, sb):
                    units.append((b, s0, sb))

        def phase1(unit):
            # HBM -> SBUF fp32 loads on the sync HWDGE queue (SWDGE issue
            # on gpsimd measured 10-40us/load), then GpSimd casts to bf16
            # (DVE would order the cast behind a whole unit's v-muls).
            # The very first unit loads in 128-row chunks so its first
            # transpose can start ~3us in instead of ~8us.
            b, s0, sz = unit
            chunk = P if (b == 0 and s0 == 0) else ld_chunk
            bts = []
            for c0 in range(0, sz, chunk):
                csz = min(chunk, sz - c0)
                it = inp.tile([P, csz // P, h], F32, name="it")
                nc.sync.dma_start(
                    it[:], enc[b, s0 + c0:s0 + c0 + csz, :].rearrange(
                        "(si p) h -> p si h", p=P))
                bt = bfp.tile([P, csz // P, h], BF16, name="bt")
                nc.gpsimd.tensor_copy(out=bt[:], in_=it[:])
                bts.append(bt)
            return bts

        def phase2(unit, bts):
            # SBUF xbar transpose [s,h] bf16 -> [h,s] on the scalar HWDGE
            # queue so the sync ring only ever carries the enc loads.
            # Returns (eT, thunks): the transposes are emitted lazily by
            # phase3_mm between its ACTIVATEs -- emitting them as one
            # block head-of-line-blocked the ACTIVATEs behind them for
            # ~5us per unit (PSUM backpressure then stalled the PE).
            b, s0, sz = unit
            eT = encT_p.tile([P, n_hc, sz], BF16, name="eT")
            thunks = []
            col = 0
            for bt in bts:
                for si in range(bt.shape[1]):
                    def t(bt=bt, si=si, col=col):
                        nc.scalar.dma_start_transpose(
                            eT[:, :, col:col + P], bt[:, si, :])
                    thunks.append(t)
                    col += P
            return eT, thunks

        def phase3_mm(unit, eT, hb, next_thunks=()):
            # PSUM matmul output must fit one 2KB bank -> 512-wide chunks.
            # Consecutive matmuls MUST alternate PSUM banks: back-to-back
            # accumulation into the same bank costs ~50ns per matmul
            # (264ns vs 213ns measured), hence `for hc: for c0`.
            # tanh on ACT, x v_o and the ot-accumulation on DVE.  The
            # next unit's transposes are sprinkled between the first
            # ACTIVATEs so they dispatch early but never head-of-line-
            # block an ACTIVATE whose PSUM bank the PE is waiting on.
            b, s0, sz = unit
            acc = acc_p.tile([P, sz], BF16, name="acc")
            tq = list(next_thunks)
            for ot in range(n_ot):
                eng = en_p.tile([P, sz], BF16, name="eng")
                pe = pe_p.tile([P, sz], F32, name="pe")
                for hc in range(n_hc):
                    for c0 in range(0, sz, 512):
                        nc.tensor.matmul(
                            pe[:, c0:c0 + 512],
                            wt_bf[:, hc, ot * P:(ot + 1) * P],
                            eT[:, hc, c0:c0 + 512],
                            start=(hc == 0), stop=(hc == n_hc - 1))
                nc.scalar.activation(
                    eng[:], pe[:], AF.Tanh, bias=hb[:, ot, b:b + 1])
                for _ in range(2):
                    if tq:
                        tq.pop(0)()
                if ot == 0:
                    nc.vector.tensor_scalar_mul(acc[:], eng[:], vt_f[:, 0:1])
                else:
                    tmp = tmp_p.tile([P, sz], BF16, name="tmp")
                    nc.vector.tensor_scalar_mul(
                        tmp[:], eng[:], vt_f[:, ot:ot + 1])
                    nc.vector.tensor_tensor(acc[:], acc[:], tmp[:], ADD)
            for t in tq:
                t()
            return acc

        def phase3_fin(unit, acc):
            # partition-reduce via ones-matmul (emitted one unit late so
            # the PE never waits on the DVE acc), then DVE copies the
            # logits PSUM->SBUF.
            b, s0, sz = unit
            for c0 in range(0, sz, 512):
                pa = pa_p.tile([P, 512], F32, name="pa", tag="pa")
                nc.tensor.matmul(
                    pa[0:1, :], ones_bf[:, 0:1], acc[:, c0:c0 + 512],
                    start=True, stop=True)
                nc.vector.tensor_copy(
                    out=att_rows[b][0:1, s0 + c0:s0 + c0 + 512],
                    in_=pa[0:1, :])

        def softmax_row(b):
            # Runs 2+ units after row b's logits landed in SBUF, so every
            # input is long ready when each queue reaches these ops.
            # |att| < ~6, so exp() is safe in fp32 without the row max.
            e_row = row_p.tile([1, s], F32, name="erow")
            ssum = const.tile([1, 1], F32, name=f"ssum{b}")
            nc.scalar.activation(
                e_row[:], att_rows[b][:], AF.Exp, accum_out=ssum[:])
            rinv = const.tile([1, 1], F32, name=f"rinv{b}")
            nc.vector.reciprocal(rinv[:], ssum[:])
            nc.vector.tensor_scalar_mul(e_row[:], e_row[:], rinv[:])
            nc.gpsimd.dma_start(out[b:b + 1, :], e_row[:])

        # ---- software pipeline, 2-unit load lookahead:
        #   iter i: matmuls(u_i) + interleaved transposes(u_{i+1})
        #           | loads+casts(u_{i+2}) | fin(u_{i-1}) | softmax
        LOOK = 2
        bts_q = {}
        bts_q[0] = phase1(units[0])
        eT_cur, t0_thunks = phase2(units[0], bts_q.pop(0))
        for t in t0_thunks:
            t()
        for k in range(1, min(LOOK, len(units))):
            bts_q[k] = phase1(units[k])

        fin = None
        sm_row = None
        for i, u in enumerate(units):
            eT_next = None
            thunks = ()
            if i + 1 < len(units):
                eT_next, thunks = phase2(units[i + 1], bts_q.pop(i + 1))
            acc = phase3_mm(u, eT_cur, hb, thunks)
            eT_cur = eT_next
            if i + LOOK < len(units):
                bts_q[i + LOOK] = phase1(units[i + LOOK])
            if sm_row is not None:
                softmax_row(sm_row)
                sm_row = None
            if fin is not None:
                phase3_fin(*fin)
                fb, fs0, fsz = fin[0]
                if fs0 + fsz == s:
                    sm_row = fb
            fin = (u, acc)
            if i == 0:
                keepalive(startup_keep)
            else:
                keepalive(keepalive_mm)
        phase3_fin(*fin)
        softmax_row(b_loc - 1)

    nc.compile()
    return nc


def make_in_maps(hidden, encoder_outputs, W_attn, b_attn, v, n_cores=8):
    hidden = np.asarray(hidden, dtype=np.float32)
    encoder_outputs = np.asarray(encoder_outputs, dtype=np.float32)
    W_attn = np.asarray(W_attn, dtype=np.float32)
    b_attn = np.asarray(b_attn, dtype=np.float32)
    v = np.asarray(v, dtype=np.float32)

    b = encoder_outputs.shape[0]
    h = W_attn.shape[0]
    b_loc = b // n_cores
    # device only needs We.T; h_proj + b_attn precomputed here (67 MFLOP)
    wt = np.ascontiguousarray(W_attn[:, h:].T.astype(ml_dtypes.bfloat16))
    hp = hidden[0] @ W_attn[:, :h].T + b_attn          # [B, h] fp32
    ones = np.ones((P, 1), dtype=ml_dtypes.bfloat16)
    zeros = np.zeros((P, 512), dtype=ml_dtypes.bfloat16)
    in_maps = []
    for i in range(n_cores):
        bsl = slice(b_loc * i, b_loc * (i + 1))
        # hbias[p, oc, b] = hp[b, oc*128 + p]
        hbias = np.ascontiguousarray(
            hp[bsl].T.reshape(h // P, P, b_loc).transpose(1, 0, 2))
        in_maps.append({
            "wt": wt,
            "hbias": hbias,
            "v": v,
            "ones": ones,
            "zeros": zeros,
            "enc": np.ascontiguousarray(encoder_outputs[bsl]),
        })
    return in_maps


_NC_CACHE = {}


def _get_nc():
    if "nc" not in _NC_CACHE:
        _NC_CACHE["nc"] = build_nc(b_loc=4, s=2048, h=1024, n_cores=8)
    return _NC_CACHE["nc"]


def kernel(hidden, encoder_outputs, W_attn, b_attn, v):
    from concourse.bass_utils import run_bass_kernel_spmd

    nc = _get_nc()
    in_maps = make_in_maps(hidden, encoder_outputs, W_attn, b_attn, v,
                           n_cores=8)
    res = run_bass_kernel_spmd(nc, in_maps, core_ids=list(range(8)))
    out = np.concatenate([np.asarray(res.results[i]["out"])
                          for i in range(8)], axis=0)
    return out.astype(np.float32)


# revision 24
# speedup vs baseline: 1.0298x; 1.0298x over previous
"""Trainium2 Bass kernel for nn_Attention_85074712199827.

Computes, for hidden [1,32,1024], encoder_outputs [32,2048,1024],
W_attn [1024,2048], b_attn [1024], v [1024]:

    h_proj  = hidden[0] @ W_attn[:, :1024].T
    e_proj  = encoder_outputs @ W_attn[:, 1024:].T
    energy  = tanh(e_proj + h_proj[:, None, :] + b_attn)
    att     = energy @ v
    out     = softmax(att, axis=1)          # [32, 2048] float32

Distribution: data-parallel over the batch across 8 NeuronCores (4
batch rows per core); the tiny parameters are replicated.  h_proj +
b_attn (67 MFLOP, 0.04% of the work) is precomputed on the host and
shipped as a [128, 8, 4] bias tensor, so the device only runs the
e_proj pipeline.  Each core runs an independent Bass/Tile program;
results are concatenated on the host.

Per-core schedule: enc streams HBM->SBUF fp32 on the sync HWDGE queue
(loads only -- nothing else shares that ring), is cast to bf16 on the
GpSimd engine (so the cast never queues behind DVE work and input
buffers recycle immediately), and xbar-transposed to [h, s] layout on
the scalar HWDGE queue, with the transposes emitted between the tanh
ACTIVATEs so they dispatch early without head-of-line-blocking them.
The e_proj matmuls alternate PSUM banks (`for hc: for c0`) -- back-to-
back accumulation into the same bank costs ~50ns per matmul.  The
v-dot is NOT done with 1-row PE matmuls: the tanh output is scaled by
v on the scalar engine, the 8 o-chunks are summed on DVE, and a single
ones-vector matmul reduces the 128 partitions.  Softmax runs per batch
row as soon as that row's attention logits are complete, overlapped
with the next row's matmuls.

Self-contained: only environment packages (concourse, numpy, ml_dtypes)
are imported; all shapes/sharding are hardcoded for this problem.
"""

from contextlib import ExitStack

import ml_dtypes
import numpy as np

import concourse.bass as bass
import concourse.tile as tile
from concourse import bacc, mybir

F32 = mybir.dt.float32
BF16 = mybir.dt.bfloat16
AF = mybir.ActivationFunctionType
ADD = mybir.AluOpType.add
P = 128


def build_nc(b_loc=4, s=2048, h=1024, n_cores=8, sb=1024,
             warmup_mm=20, startup_keep=4, keepalive_mm=0,
             first_sb=512, ld_chunk=512):
    n_hc = h // P           # contraction chunks
    n_ot = h // P           # output (o) tiles

    nc = bacc.Bacc("TRN2", target_bir_lowering=False, debug=False,
                   num_devices=n_cores)

    wt = nc.dram_tensor("wt", [h, h], BF16, kind="ExternalInput").ap()
    hbias = nc.dram_tensor("hbias", [P, n_ot, b_loc], F32,
                           kind="ExternalInput").ap()
    v = nc.dram_tensor("v", [h], F32, kind="ExternalInput").ap()
    ones = nc.dram_tensor("ones", [P, 1], BF16, kind="ExternalInput").ap()
    zeros = nc.dram_tensor("zeros", [P, 512], BF16, kind="ExternalInput").ap()
    enc = nc.dram_tensor("enc", [b_loc, s, h], F32, kind="ExternalInput").ap()
    out = nc.dram_tensor("out", [b_loc, s], F32, kind="ExternalOutput").ap()

    with tile.TileContext(nc) as tc, ExitStack() as ctx:
        const = ctx.enter_context(tc.tile_pool(name="const", bufs=1))
        psmall = ctx.enter_context(tc.tile_pool(name="psmall", bufs=1, space="PSUM"))

        # ---- PE warmup: dependency-free matmuls to lift the HAM clock
        # gate to 8/8 while the first enc block is still in flight ----
        wz = const.tile([P, 512], BF16)
        nc.scalar.dma_start(wz[:], zeros)
        for i in range(warmup_mm):
            pw = psmall.tile([P, 512], F32, name="pw", tag="ps")
            nc.tensor.matmul(pw[:], wz[:, :P], wz[:], start=True, stop=True)

        def keepalive(n):
            for _ in range(n):
                pw = psmall.tile([P, 512], F32, name="pw", tag="ps")
                nc.tensor.matmul(pw[:], wz[:, :P], wz[:], start=True, stop=True)

        # ---- weights: We.T arrives [h, h] bf16, streamed first so the
        # first e_proj matmuls unblock asap; then the small constants ----
        wt_bf = const.tile([P, n_hc, h], BF16)
        wt_r = wt.rearrange("(jc p) o -> p jc o", p=P)
        q = n_hc // 2
        for c in range(2):
            nc.scalar.dma_start(
                wt_bf[:, c * q:(c + 1) * q, :],
                wt_r[:, c * q:(c + 1) * q, :])

        hb = const.tile([P, n_ot, b_loc], F32, name="hb")
        nc.scalar.dma_start(hb[:], hbias)

        vt_f = const.tile([P, n_ot], F32)
        nc.scalar.dma_start(vt_f[:], v.rearrange("(oc p) -> p oc", p=P))

        ones_bf = const.tile([P, 1], BF16)
        nc.scalar.dma_start(ones_bf[:], ones)

        # ---- main pipeline pools ----
        inp = ctx.enter_context(tc.tile_pool(name="inp", bufs=4))
        bfp = ctx.enter_context(tc.tile_pool(name="bfp", bufs=4))
        encT_p = ctx.enter_context(tc.tile_pool(name="encT", bufs=2))
        en_p = ctx.enter_context(tc.tile_pool(name="energy", bufs=3))
        tmp_p = ctx.enter_context(tc.tile_pool(name="vtmp", bufs=3))
        acc_p = ctx.enter_context(tc.tile_pool(name="acc", bufs=3))
        row_p = ctx.enter_context(tc.tile_pool(name="rowbuf", bufs=1))
        pe_p = ctx.enter_context(tc.tile_pool(name="psum_e", bufs=2, space="PSUM"))
        pa_p = ctx.enter_context(tc.tile_pool(name="psum_att", bufs=2, space="PSUM"))

        # per-row logits buffers (DVE/ACT accesses must start at partition 0)
        att_rows = [const.tile([1, s], F32, name=f"attrow{b}")
                    for b in range(b_loc)]

        # units: (b, s0, size) — b-major so softmax(b) pipelines.
        # The first units of b=0 are small for a fast pipeline rampup.
        units = []
        for b in range(b_loc):
            if b == 0 and first_sb < sb:
                for s0 in range(0, sb, first_sb):
                    units.append((b, s0, first_sb))
                for s0 in range(sb, s, sb):
                    units.append((b, s0, sb))
            else:
                for s0 in range(0, s, sb):
                    units.append((b, s0, sb))

        def phase1a(unit):
            # HBM -> SBUF fp32 loads on the sync HWDGE queue, issued TWO
            # units before their casts so the casts never wait.
            b, s0, sz = unit
            chunk = P if (b == 0 and s0 == 0) else ld_chunk
            its = []
            for c0 in range(0, sz, chunk):
                csz = min(chunk, sz - c0)
                it = inp.tile([P, csz // P, h], F32, name="it")
                nc.sync.dma_start(
                    it[:], enc[b, s0 + c0:s0 + c0 + csz, :].rearrange(
                        "(si p) h -> p si h", p=P))
                its.append(it)
            return its

        def phase1b(its):
            # DVE casts fp32 -> bf16, emitted at the HEAD of an
            # iteration's DVE stream: their loads completed an iteration
            # ago, so the DVE never blocks on them, and putting them
            # last (behind the v-muls, which pace to the iteration's
            # end) made the next unit's transposes 10us+ late.
            bts = []
            for it in its:
                bt = bfp.tile([P, it.shape[1], h], BF16, name="bt")
                nc.vector.tensor_copy(out=bt[:], in_=it[:])
                bts.append(bt)
            return bts

        def phase2(unit, bts):
            # SBUF xbar transpose [s,h] bf16 -> [h,s] on the scalar HWDGE
            # queue so the sync ring only ever carries the enc loads.
            # Returns (eT, thunks): the transposes are emitted lazily by
            # phase3_mm between its ACTIVATEs -- emitting them as one
            # block head-of-line-blocked the ACTIVATEs behind them for
            # ~5us per unit (PSUM backpressure then stalled the PE).
            b, s0, sz = unit
            eT = encT_p.tile([P, n_hc, sz], BF16, name="eT")
            thunks = []
            col = 0
            for bt in bts:
                for si in range(bt.shape[1]):
                    def t(bt=bt, si=si, col=col):
                        nc.scalar.dma_start_transpose(
                            eT[:, :, col:col + P], bt[:, si, :])
                    thunks.append(t)
                    col += P
            return eT, thunks

        def phase3_mm(unit, eT, hb, next_thunks=()):
            # PSUM matmul output must fit one 2KB bank -> 512-wide chunks.
            # Consecutive matmuls MUST alternate PSUM banks: back-to-back
            # accumulation into the same bank costs ~50ns per matmul
            # (264ns vs 213ns measured), hence `for hc: for c0`.
            # tanh on ACT, x v_o and the ot-accumulation on DVE.  The
            # next unit's transposes are sprinkled between the first
            # ACTIVATEs so they dispatch early but never head-of-line-
            # block an ACTIVATE whose PSUM bank the PE is waiting on.
            b, s0, sz = unit
            acc = acc_p.tile([P, sz], BF16, name="acc")
            tq = list(next_thunks)
            for ot in range(n_ot):
                eng = en_p.tile([P, sz], BF16, name="eng")
                pe = pe_p.tile([P, sz], F32, name="pe")
                for hc in range(n_hc):
                    for c0 in range(0, sz, 512):
                        nc.tensor.matmul(
                            pe[:, c0:c0 + 512],
                            wt_bf[:, hc, ot * P:(ot + 1) * P],
                            eT[:, hc, c0:c0 + 512],
                            start=(hc == 0), stop=(hc == n_hc - 1))
                nc.scalar.activation(
                    eng[:], pe[:], AF.Tanh, bias=hb[:, ot, b:b + 1])
                for _ in range(2):
                    if tq:
                        tq.pop(0)()
                if ot == 0:
                    nc.vector.tensor_scalar_mul(acc[:], eng[:], vt_f[:, 0:1])
                else:
                    tmp = tmp_p.tile([P, sz], BF16, name="tmp")
                    nc.vector.tensor_scalar_mul(
                        tmp[:], eng[:], vt_f[:, ot:ot + 1])
                    nc.vector.tensor_tensor(acc[:], acc[:], tmp[:], ADD)
            for t in tq:
                t()
            return acc

        def phase3_fin(unit, acc):
            # partition-reduce via ones-matmul (emitted one unit late so
            # the PE never waits on the DVE acc), then DVE copies the
            # logits PSUM->SBUF.
            b, s0, sz = unit
            for c0 in range(0, sz, 512):
                pa = pa_p.tile([P, 512], F32, name="pa", tag="pa")
                nc.tensor.matmul(
                    pa[0:1, :], ones_bf[:, 0:1], acc[:, c0:c0 + 512],
                    start=True, stop=True)
                nc.vector.tensor_copy(
                    out=att_rows[b][0:1, s0 + c0:s0 + c0 + 512],
                    in_=pa[0:1, :])

        def softmax_row(b):
            # Runs 2+ units after row b's logits landed in SBUF, so every
            # input is long ready when each queue reaches these ops.
            # |att| < ~6, so exp() is safe in fp32 without the row max.
            e_row = row_p.tile([1, s], F32, name="erow")
            ssum = const.tile([1, 1], F32, name=f"ssum{b}")
            nc.scalar.activation(
                e_row[:], att_rows[b][:], AF.Exp, accum_out=ssum[:])
            rinv = const.tile([1, 1], F32, name=f"rinv{b}")
            nc.vector.reciprocal(rinv[:], ssum[:])
            nc.vector.tensor_scalar_mul(e_row[:], e_row[:], rinv[:])
            nc.gpsimd.dma_start(out[b:b + 1, :], e_row[:])

        # ---- software pipeline:
        #   iter i: casts(u_{i+2}) at the DVE head | matmuls(u_i) with
        #           transposes(u_{i+1}) interleaved | loads(u_{i+3}) |
        #           fin(u_{i-1}) | softmax(row completed at u_{i-2})
        n_u = len(units)
        its_q = {}
        bts_q = {}
        its_q[0] = phase1a(units[0])
        bts_q[0] = phase1b(its_q.pop(0))
        eT_cur, t0_thunks = phase2(units[0], bts_q.pop(0))
        for t in t0_thunks:
            t()
        for k in (1, 2):
            if k < n_u:
                its_q[k] = phase1a(units[k])
        if 1 in its_q:
            bts_q[1] = phase1b(its_q.pop(1))

        fin = None
        sm_row = None
        for i, u in enumerate(units):
            if i + 2 in its_q:
                bts_q[i + 2] = phase1b(its_q.pop(i + 2))
            eT_next = None
            thunks = ()
            if i + 1 < n_u:
                eT_next, thunks = phase2(units[i + 1], bts_q.pop(i + 1))
            acc = phase3_mm(u, eT_cur, hb, thunks)
            eT_cur = eT_next
            if i + 3 < n_u:
                its_q[i + 3] = phase1a(units[i + 3])
            if sm_row is not None:
                softmax_row(sm_row)
                sm_row = None
            if fin is not None:
                phase3_fin(*fin)
                fb, fs0, fsz = fin[0]
                if fs0 + fsz == s:
                    sm_row = fb
            fin = (u, acc)
            if i == 0:
                keepalive(startup_keep)
            else:
                keepalive(keepalive_mm)
        phase3_fin(*fin)
        softmax_row(b_loc - 1)

    nc.compile()
    return nc


def make_in_maps(hidden, encoder_outputs, W_attn, b_attn, v, n_cores=8):
    hidden = np.asarray(hidden, dtype=np.float32)
    encoder_outputs = np.asarray(encoder_outputs, dtype=np.float32)
    W_attn = np.asarray(W_attn, dtype=np.float32)
    b_attn = np.asarray(b_attn, dtype=np.float32)
    v = np.asarray(v, dtype=np.float32)

    b = encoder_outputs.shape[0]
    h = W_attn.shape[0]
    b_loc = b // n_cores
    # device only needs We.T; h_proj + b_attn precomputed here (67 MFLOP)
    wt = np.ascontiguousarray(W_attn[:, h:].T.astype(ml_dtypes.bfloat16))
    hp = hidden[0] @ W_attn[:, :h].T + b_attn          # [B, h] fp32
    ones = np.ones((P, 1), dtype=ml_dtypes.bfloat16)
    zeros = np.zeros((P, 512), dtype=ml_dtypes.bfloat16)
    in_maps = []
    for i in range(n_cores):
        bsl = slice(b_loc * i, b_loc * (i + 1))
        # hbias[p, oc, b] = hp[b, oc*128 + p]
        hbias = np.ascontiguousarray(
            hp[bsl].T.reshape(h // P, P, b_loc).transpose(1, 0, 2))
        in_maps.append({
            "wt": wt,
            "hbias": hbias,
            "v": v,
            "ones": ones,
            "zeros": zeros,
            "enc": np.ascontiguousarray(encoder_outputs[bsl]),
        })
    return in_maps


_NC_CACHE = {}


def _get_nc():
    if "nc" not in _NC_CACHE:
        _NC_CACHE["nc"] = build_nc(b_loc=4, s=2048, h=1024, n_cores=8)
    return _NC_CACHE["nc"]


def kernel(hidden, encoder_outputs, W_attn, b_attn, v):
    from concourse.bass_utils import run_bass_kernel_spmd

    nc = _get_nc()
    in_maps = make_in_maps(hidden, encoder_outputs, W_attn, b_attn, v,
                           n_cores=8)
    res = run_bass_kernel_spmd(nc, in_maps, core_ids=list(range(8)))
    out = np.concatenate([np.asarray(res.results[i]["out"])
                          for i in range(8)], axis=0)
    return out.astype(np.float32)


# revision 27
# speedup vs baseline: 1.1589x; 1.1254x over previous
"""Trainium2 Bass kernel for nn_Attention_85074712199827.

Computes, for hidden [1,32,1024], encoder_outputs [32,2048,1024],
W_attn [1024,2048], b_attn [1024], v [1024]:

    h_proj  = hidden[0] @ W_attn[:, :1024].T
    e_proj  = encoder_outputs @ W_attn[:, 1024:].T
    energy  = tanh(e_proj + h_proj[:, None, :] + b_attn)
    att     = energy @ v
    out     = softmax(att, axis=1)          # [32, 2048] float32

Distribution: data-parallel over the batch across 8 NeuronCores (4
batch rows per core); the tiny parameters are replicated.  h_proj +
b_attn (67 MFLOP, 0.04% of the work) is precomputed on the host and
shipped as a [128, 8, 4] bias tensor, so the device only runs the
e_proj pipeline.  Each core runs an independent Bass/Tile program;
results are concatenated on the host.

Per-core schedule: enc streams HBM->SBUF fp32 on the sync HWDGE queue
(loads only -- nothing else shares that ring), is cast to bf16 on the
GpSimd engine (so the cast never queues behind DVE work and input
buffers recycle immediately), and xbar-transposed to [h, s] layout on
the scalar HWDGE queue, with the transposes emitted between the tanh
ACTIVATEs so they dispatch early without head-of-line-blocking them.
The e_proj matmuls alternate PSUM banks (`for hc: for c0`) -- back-to-
back accumulation into the same bank costs ~50ns per matmul.  The
v-dot is NOT done with 1-row PE matmuls: the tanh output is scaled by
v on the scalar engine, the 8 o-chunks are summed on DVE, and a single
ones-vector matmul reduces the 128 partitions.  Softmax runs per batch
row as soon as that row's attention logits are complete, overlapped
with the next row's matmuls.

Self-contained: only environment packages (concourse, numpy, ml_dtypes)
are imported; all shapes/sharding are hardcoded for this problem.
"""

from contextlib import ExitStack

import ml_dtypes
import numpy as np

import concourse.bass as bass
import concourse.tile as tile
from concourse import bacc, mybir

F32 = mybir.dt.float32
BF16 = mybir.dt.bfloat16
AF = mybir.ActivationFunctionType
ADD = mybir.AluOpType.add
P = 128


def build_nc(b_loc=4, s=2048, h=1024, n_cores=8, sb=1024,
             warmup_mm=14, startup_keep=4, keepalive_mm=0,
             first_sb=512, ld_chunk=512):
    n_hc = h // P           # contraction chunks
    n_ot = h // P           # output (o) tiles

    nc = bacc.Bacc("TRN2", target_bir_lowering=False, debug=False,
                   num_devices=n_cores)

    wt = nc.dram_tensor("wt", [h, h], BF16, kind="ExternalInput").ap()
    hbias = nc.dram_tensor("hbias", [P, n_ot, b_loc], F32,
                           kind="ExternalInput").ap()
    v = nc.dram_tensor("v", [h], F32, kind="ExternalInput").ap()
    ones = nc.dram_tensor("ones", [P, 1], BF16, kind="ExternalInput").ap()
    zeros = nc.dram_tensor("zeros", [P, 512], BF16, kind="ExternalInput").ap()
    enc = nc.dram_tensor("enc", [b_loc, s, h], F32, kind="ExternalInput").ap()
    out = nc.dram_tensor("out", [b_loc, s], F32, kind="ExternalOutput").ap()

    with tile.TileContext(nc) as tc, ExitStack() as ctx:
        const = ctx.enter_context(tc.tile_pool(name="const", bufs=1))
        psmall = ctx.enter_context(tc.tile_pool(name="psmall", bufs=1, space="PSUM"))

        # ---- PE warmup: dependency-free matmuls to lift the HAM clock
        # gate to 8/8 while the first enc block is still in flight ----
        wz = const.tile([P, 512], BF16)
        nc.scalar.dma_start(wz[:], zeros)
        for i in range(warmup_mm):
            pw = psmall.tile([P, 512], F32, name="pw", tag="ps")
            nc.tensor.matmul(pw[:], wz[:, :P], wz[:], start=True, stop=True)

        def keepalive(n):
            for _ in range(n):
                pw = psmall.tile([P, 512], F32, name="pw", tag="ps")
                nc.tensor.matmul(pw[:], wz[:, :P], wz[:], start=True, stop=True)

        # ---- weights/constants: emitted around the first transposes in
        # the order each is first needed (scalar-ring DMAs complete
        # serially at ~2-3us each, so order = arrival time) ----
        wt_bf = const.tile([P, n_hc, h], BF16)
        wt_r = wt.rearrange("(jc p) o -> p jc o", p=P)
        q = n_hc // 2

        def emit_w(c):
            nc.scalar.dma_start(
                wt_bf[:, c * q:(c + 1) * q, :],
                wt_r[:, c * q:(c + 1) * q, :])

        hb = const.tile([P, n_ot, b_loc], F32, name="hb")
        vt_f = const.tile([P, n_ot], F32)
        ones_bf = const.tile([P, 1], BF16)

        def emit_consts():
            nc.scalar.dma_start(hb[:], hbias)
            nc.scalar.dma_start(vt_f[:], v.rearrange("(oc p) -> p oc", p=P))
            nc.scalar.dma_start(ones_bf[:], ones)

        # ---- main pipeline pools ----
        inp = ctx.enter_context(tc.tile_pool(name="inp", bufs=4))
        bfp = ctx.enter_context(tc.tile_pool(name="bfp", bufs=4))
        encT_p = ctx.enter_context(tc.tile_pool(name="encT", bufs=2))
        en_p = ctx.enter_context(tc.tile_pool(name="energy", bufs=3))
        tmp_p = ctx.enter_context(tc.tile_pool(name="vtmp", bufs=3))
        acc_p = ctx.enter_context(tc.tile_pool(name="acc", bufs=3))
        row_p = ctx.enter_context(tc.tile_pool(name="rowbuf", bufs=1))
        pe_p = ctx.enter_context(tc.tile_pool(name="psum_e", bufs=2, space="PSUM"))
        pa_p = ctx.enter_context(tc.tile_pool(name="psum_att", bufs=2, space="PSUM"))

        # per-row logits buffers (DVE/ACT accesses must start at partition 0)
        att_rows = [const.tile([1, s], F32, name=f"attrow{b}")
                    for b in range(b_loc)]

        # units: (b, s0, size) — b-major so softmax(b) pipelines.
        # The first units of b=0 are small for a fast pipeline rampup.
        units = []
        for b in range(b_loc):
            if b == 0 and first_sb < sb:
                for s0 in range(0, sb, first_sb):
                    units.append((b, s0, first_sb))
                for s0 in range(sb, s, sb):
                    units.append((b, s0, sb))
            else:
                for s0 in range(0, s, sb):
                    units.append((b, s0, sb))

        def phase1a(unit):
            # HBM -> SBUF fp32 loads on the sync HWDGE queue, issued TWO
            # units before their casts so the casts never wait.
            b, s0, sz = unit
            chunk = P if (b == 0 and s0 == 0) else ld_chunk
            its = []
            for c0 in range(0, sz, chunk):
                csz = min(chunk, sz - c0)
                it = inp.tile([P, csz // P, h], F32, name="it")
                nc.sync.dma_start(
                    it[:], enc[b, s0 + c0:s0 + c0 + csz, :].rearrange(
                        "(si p) h -> p si h", p=P))
                its.append(it)
            return its

        def phase1b(its):
            # DVE casts fp32 -> bf16, emitted at the HEAD of an
            # iteration's DVE stream: their loads completed an iteration
            # ago, so the DVE never blocks on them, and putting them
            # last (behind the v-muls, which pace to the iteration's
            # end) made the next unit's transposes 10us+ late.
            bts = []
            for it in its:
                bt = bfp.tile([P, it.shape[1], h], BF16, name="bt")
                nc.vector.tensor_copy(out=bt[:], in_=it[:])
                bts.append(bt)
            return bts

        def phase2(unit, bts):
            # SBUF xbar transpose [s,h] bf16 -> [h,s] on the scalar HWDGE
            # queue so the sync ring only ever carries the enc loads.
            # Returns (eT, thunks): the transposes are emitted lazily by
            # phase3_mm between its ACTIVATEs -- emitting them as one
            # block head-of-line-blocked the ACTIVATEs behind them for
            # ~5us per unit (PSUM backpressure then stalled the PE).
            b, s0, sz = unit
            eT = encT_p.tile([P, n_hc, sz], BF16, name="eT")
            thunks = []
            col = 0
            for bt in bts:
                for si in range(bt.shape[1]):
                    def t(bt=bt, si=si, col=col):
                        nc.scalar.dma_start_transpose(
                            eT[:, :, col:col + P], bt[:, si, :])
                    thunks.append(t)
                    col += P
            return eT, thunks

        def phase3_mm(unit, eT, hb, next_thunks=()):
            # PSUM matmul output must fit one 2KB bank -> 512-wide chunks.
            # Consecutive matmuls MUST alternate PSUM banks: back-to-back
            # accumulation into the same bank costs ~50ns per matmul
            # (264ns vs 213ns measured), hence `for hc: for c0`.
            # tanh on ACT, x v_o and the ot-accumulation on DVE.  The
            # next unit's transposes are sprinkled between the first
            # ACTIVATEs so they dispatch early but never head-of-line-
            # block an ACTIVATE whose PSUM bank the PE is waiting on.
            b, s0, sz = unit
            acc = acc_p.tile([P, sz], BF16, name="acc")
            tq = list(next_thunks)
            for ot in range(n_ot):
                eng = en_p.tile([P, sz], BF16, name="eng")
                pe = pe_p.tile([P, sz], F32, name="pe")
                for hc in range(n_hc):
                    for c0 in range(0, sz, 512):
                        nc.tensor.matmul(
                            pe[:, c0:c0 + 512],
                            wt_bf[:, hc, ot * P:(ot + 1) * P],
                            eT[:, hc, c0:c0 + 512],
                            start=(hc == 0), stop=(hc == n_hc - 1))
                nc.scalar.activation(
                    eng[:], pe[:], AF.Tanh, bias=hb[:, ot, b:b + 1])
                for _ in range(2):
                    if tq:
                        tq.pop(0)()
                if ot == 0:
                    nc.vector.tensor_scalar_mul(acc[:], eng[:], vt_f[:, 0:1])
                else:
                    tmp = tmp_p.tile([P, sz], BF16, name="tmp")
                    nc.vector.tensor_scalar_mul(
                        tmp[:], eng[:], vt_f[:, ot:ot + 1])
                    nc.vector.tensor_tensor(acc[:], acc[:], tmp[:], ADD)
            for t in tq:
                t()
            return acc

        def phase3_fin(unit, acc):
            # partition-reduce via ones-matmul (emitted one unit late so
            # the PE never waits on the DVE acc), then DVE copies the
            # logits PSUM->SBUF.
            b, s0, sz = unit
            for c0 in range(0, sz, 512):
                pa = pa_p.tile([P, 512], F32, name="pa", tag="pa")
                nc.tensor.matmul(
                    pa[0:1, :], ones_bf[:, 0:1], acc[:, c0:c0 + 512],
                    start=True, stop=True)
                nc.vector.tensor_copy(
                    out=att_rows[b][0:1, s0 + c0:s0 + c0 + 512],
                    in_=pa[0:1, :])

        def softmax_row(b):
            # Runs 2+ units after row b's logits landed in SBUF, so every
            # input is long ready when each queue reaches these ops.
            # |att| < ~6, so exp() is safe in fp32 without the row max.
            e_row = row_p.tile([1, s], F32, name="erow")
            ssum = const.tile([1, 1], F32, name=f"ssum{b}")
            nc.scalar.activation(
                e_row[:], att_rows[b][:], AF.Exp, accum_out=ssum[:])
            rinv = const.tile([1, 1], F32, name=f"rinv{b}")
            nc.vector.reciprocal(rinv[:], ssum[:])
            nc.vector.tensor_scalar_mul(e_row[:], e_row[:], rinv[:])
            nc.gpsimd.dma_start(out[b:b + 1, :], e_row[:])

        # ---- software pipeline:
        #   iter i: casts(u_{i+1}) at the DVE head (their loads landed
        #           during iter i-1) | matmuls(u_i) with transposes
        #           (u_{i+1}) interleaved | loads(u_{i+2}) |
        #           fin(u_{i-1}) | softmax(row completed at u_{i-2})
        # Startup scalar-ring order: zeros, Wc0, T(u0)x4, Wc1, hb, vt,
        # ones — each arrives just before its first consumer needs it.
        n_u = len(units)
        its_q = {}
        its_q[0] = phase1a(units[0])
        emit_w(0)
        bts0 = phase1b(its_q.pop(0))
        eT_cur, t0_thunks = phase2(units[0], bts0)
        for t in t0_thunks:
            t()
        emit_w(1)
        emit_consts()
        if 1 < n_u:
            its_q[1] = phase1a(units[1])

        fin = None
        sm_row = None
        for i, u in enumerate(units):
            eT_next = None
            thunks = ()
            if i + 1 < n_u:
                bts = phase1b(its_q.pop(i + 1))
                eT_next, thunks = phase2(units[i + 1], bts)
            acc = phase3_mm(u, eT_cur, hb, thunks)
            eT_cur = eT_next
            if i + 2 < n_u:
                its_q[i + 2] = phase1a(units[i + 2])
            if sm_row is not None:
                softmax_row(sm_row)
                sm_row = None
            if fin is not None:
                phase3_fin(*fin)
                fb, fs0, fsz = fin[0]
                if fs0 + fsz == s:
                    sm_row = fb
            fin = (u, acc)
            if i == 0:
                keepalive(startup_keep)
            else:
                keepalive(keepalive_mm)
        phase3_fin(*fin)
        softmax_row(b_loc - 1)

    nc.compile()
    return nc


def make_in_maps(hidden, encoder_outputs, W_attn, b_attn, v, n_cores=8):
    hidden = np.asarray(hidden, dtype=np.float32)
    encoder_outputs = np.asarray(encoder_outputs, dtype=np.float32)
    W_attn = np.asarray(W_attn, dtype=np.float32)
    b_attn = np.asarray(b_attn, dtype=np.float32)
    v = np.asarray(v, dtype=np.float32)

    b = encoder_outputs.shape[0]
    h = W_attn.shape[0]
    b_loc = b // n_cores
    # device only needs We.T; h_proj + b_attn precomputed here (67 MFLOP)
    wt = np.ascontiguousarray(W_attn[:, h:].T.astype(ml_dtypes.bfloat16))
    hp = hidden[0] @ W_attn[:, :h].T + b_attn          # [B, h] fp32
    ones = np.ones((P, 1), dtype=ml_dtypes.bfloat16)
    zeros = np.zeros((P, 512), dtype=ml_dtypes.bfloat16)
    in_maps = []
    for i in range(n_cores):
        bsl = slice(b_loc * i, b_loc * (i + 1))
        # hbias[p, oc, b] = hp[b, oc*128 + p]
        hbias = np.ascontiguousarray(
            hp[bsl].T.reshape(h // P, P, b_loc).transpose(1, 0, 2))
        in_maps.append({
            "wt": wt,
            "hbias": hbias,
            "v": v,
            "ones": ones,
            "zeros": zeros,
            "enc": np.ascontiguousarray(encoder_outputs[bsl]),
        })
    return in_maps


_NC_CACHE = {}


def _get_nc():
    if "nc" not in _NC_CACHE:
        _NC_CACHE["nc"] = build_nc(b_loc=4, s=2048, h=1024, n_cores=8)
    return _NC_CACHE["nc"]


def kernel(hidden, encoder_outputs, W_attn, b_attn, v):
    from concourse.bass_utils import run_bass_kernel_spmd

    nc = _get_nc()
    in_maps = make_in_maps(hidden, encoder_outputs, W_attn, b_attn, v,
                           n_cores=8)
    res = run_bass_kernel_spmd(nc, in_maps, core_ids=list(range(8)))
    out = np.concatenate([np.asarray(res.results[i]["out"])
                          for i in range(8)], axis=0)
    return out.astype(np.float32)
